# revision 51
# baseline (speedup 1.0000x reference)
"""Trainium2 Bass kernel for the CapsuleLayer routing problem.

Strategy (pure data-parallel over 8 NeuronCores, batch-sharded):
  u = x @ W  via a 3-term fp16 hi/lo split on the TensorEngine
  (xh@Wh + xh@Wl + xl@Wh, fp32 PSUM accumulate).  fp16 pairs carry
  ~22 mantissa bits; the routing softmax amplifies matmul error
  ~1000-2000x at the batch tail, so nothing cheaper passes the 2e-2
  gate (measured: 2-term 0.36-0.62, f32r 1/2-pass 0.34-0.43,
  fp16-stored-u 0.49).

  Routing uses a custom fused DVE op (CAPS_MAC_SCAN: running prefix
  sum of Src0*Src1); segment sums are diffs of the prefix at segment
  boundaries (GPSIMD).  The software pipeline is queue-driven and 3
  groups deep: each matmul window (2.72us) hosts the 4 scans (2.38us)
  of three different groups: q1(g), s2'/q2'(g-1, with a one-tile skew
  so the Pool diff latency hides), s3'(g-2).  Beta chains are split
  into window-spaced steps (pendingB) so the in-order engines never
  wait on a cross-engine round trip; softmax max-subtraction is
  hoisted before gamma (exp(gam1*(q1-m2q)) via per-tile Act
  activations with AP scale — valid since gam1 >= 0), and nu1/sig3
  come free from Act Square+accumulator ops, so gamma1 is ready
  before the q1 scans even finish.

  Engine assignment (per-core busy at 118us total): DVE 90us = scans
  + reduces + tiny chain ops; PE 90us = matmuls (+ a sized p-state
  warm-up bridging the initial DMas); Act 76us = PSUM copies +
  exp/ln/square; GPSIMD 36us = segment diffs + big [P,GS*16/32]
  tensor ops.  Group ladder [2,3,4,5,6,5,4,2,1] primes and drains
  the pipeline; head DMAs are ordered xh(g0), W0, xl(g0) because
  DMA-descriptor generation (~625ns each) and the transfers both
  serialize; the last two groups share one output buffer and a single
  final DMA for the same reason.

Layout: batch rows on partitions, features (16 caps x 32 dims) on the
free dim.  x is staged transposed+tiled from the host so each x-tile is
directly usable as the matmul stationary operand (lhsT).
"""

import sys
import os

for _p in ("/opt/trn_rl_repo", "/root/.axon_site/_ro/trn_rl_repo"):
    if os.path.isdir(_p) and _p not in sys.path:
        sys.path.insert(0, _p)
        break

import numpy as np
import ml_dtypes

import concourse.bass as bass
import concourse.bacc as bacc
import concourse.mybir as mybir
from concourse import tile
from concourse import dve_ops as dops
from concourse.dve_spec import Spec, Src0, Src1, scan, lower, AluOp
from concourse.dve_uop import DveOpSpec
from concourse.dve_ops import DveOp
from concourse.bass_utils import run_bass_kernel_spmd

BF16 = mybir.dt.bfloat16
F16 = mybir.dt.float16
F32 = mybir.dt.float32
NP_BF16 = ml_dtypes.bfloat16

NCORES = 8
B = 32768
K = 512
CAPS = 16
D = 32
ND = CAPS * D          # 512
BS = B // NCORES       # 4096 rows per core
P = 128                # partitions per tile
TILES = BS // P        # 32
_LADDER = [2, 3, 4, 5, 6, 5, 4, 2, 1]   # tuned group-size ladder
GROUPS = []
_t0 = 0
for _gs in _LADDER:
    GROUPS.append((_t0, _gs))
    _t0 += _gs
assert _t0 == 32
KCH = K // P           # 4 contraction chunks
NSLOT = 10             # prefix-buffer ring depth

AX = mybir.AxisListType.X
OP_ADD = mybir.AluOpType.add
OP_SUB = mybir.AluOpType.subtract
OP_MUL = mybir.AluOpType.mult
OP_MAX = mybir.AluOpType.max
FN = mybir.ActivationFunctionType


def _patch_act_tables():
    """Make the act-table-load pass resolve Exp and Ln to the combined
    natural_log_exp_and_others set so one table load serves the whole
    kernel (first-fit would otherwise alternate exp<->ln sets, ~2.7us per
    switch).  Indices must stay aligned with act_info.json, so only the
    function-membership sets are edited."""
    from concourse import hw_specs
    if getattr(hw_specs, "_capsule_patched", False):
        return
    orig = hw_specs.get_activation_tables

    def patched(module_arch):
        tables = {k: set(v) for k, v in orig(module_arch).items()}
        comb = "natural_log_exp_and_others"
        if comb in tables:
            confined = [FN.Exp, FN.Ln]
            for fname in ("Square", "Copy", "Identity"):
                fn = getattr(FN, fname, None)
                if fn is not None and fn in tables[comb]:
                    confined.append(fn)
            for name, fns in tables.items():
                if name != comb:
                    for fn in confined:
                        fns.discard(fn)
        return tables

    import functools
    patched_cached = functools.cache(patched)
    hw_specs.get_activation_tables = patched_cached
    bacc.get_activation_tables = patched_cached
    hw_specs._capsule_patched = True


def _mac_scan_ref(in0, in1, c0, c1, c2):
    a = np.asarray(in0, np.float32) * np.asarray(in1, np.float32)
    flat = a.reshape(a.shape[0], -1)
    out = np.cumsum(flat.astype(np.float64), axis=1).astype(np.float32)
    return out.reshape(a.shape)


def _register_mac_scan():
    """Register the fused multiply+prefix-sum custom DVE op (documented
    per-NEFF DVE-table mechanism; one added OPS row)."""
    name = "CAPS_MAC_SCAN"
    for op in dops.OPS:
        if op.name == name:
            return op
    spec = Spec(body=scan(AluOp.ADD, Src0 * Src1), reference=_mac_scan_ref)
    row = dops._CUSTOM_DVE_ROW_BASE + len(dops.OPS)
    assert row < 0x20, "custom DVE opcode rows exhausted"
    shas = {}
    for ver in ("v3", "v4"):
        s = DveOpSpec(name=name, opcode=row, uops=lower(spec, ver=ver),
                      rd1_en=True)
        shas[ver] = s.sha(ver)
    op = DveOp(name, spec, subdim=False, uops_sha=shas)
    dops.OPS.append(op)
    dops.CUSTOM_DVE_SPECS[name] = spec
    dops._SUB_OPCODE_FOR_NAME[name] = row
    return op


def _build_program():
    _patch_act_tables()
    mac_scan = _register_mac_scan()
    nc = bacc.Bacc("TRN2", target_bir_lowering=False)

    xTh = nc.declare_dram_parameter("xh", [TILES, P, K], F16, isOutput=False)
    xTl = nc.declare_dram_parameter("xl", [TILES, P, K], F16, isOutput=False)
    WPK = 2 * (ND + D)  # 1088 packed weight cols per chunk
    Wpk = nc.declare_dram_parameter("Wpk", [KCH, P, WPK], F16, isOutput=False)
    vout = nc.declare_dram_parameter("v", [BS, D], F32, isOutput=True)
    vview = vout.ap().rearrange("(t p) d -> t p d", p=P)

    with tile.TileContext(nc) as tc:
        with (
            tc.tile_pool(name="wpool", bufs=1) as wpool,
            tc.tile_pool(name="xpool", bufs=4) as xpool,
            tc.tile_pool(name="upsum", bufs=4, space="PSUM") as upsum,
            tc.tile_pool(name="spsum", bufs=4, space="PSUM") as spsum,
            tc.tile_pool(name="upool", bufs=24) as upool,
            tc.tile_pool(name="prefpool", bufs=1) as prefpool,
            tc.tile_pool(name="s1pool", bufs=16) as s1pool,
            tc.tile_pool(name="gpool", bufs=5) as gpool,
            tc.tile_pool(name="spool", bufs=16) as spool,
        ):
            # --- constants ---
            wchunks = []

            def issue_dma(st):
                """Prefetch the group's split x tiles (two DMAs)."""
                T0, GS = st["T0"], st["GS"]
                xgh = st["xgh"] = xpool.tile([P, GS * K], F16, tag="xgh",
                                             name="xgh")
                nc.sync.dma_start(
                    xgh[:].rearrange("p (t f) -> p t f", t=GS),
                    xTh[T0:T0 + GS].rearrange("t p f -> p t f"),
                )
                xgl = st["xgl"] = xpool.tile([P, GS * K], F16, tag="xgl",
                                             name="xgl")
                nc.sync.dma_start(
                    xgl[:].rearrange("p (t f) -> p t f", t=GS),
                    xTl[T0:T0 + GS].rearrange("t p f -> p t f"),
                )

            states = [{"T0": T0, "GS": GS} for (T0, GS) in GROUPS]
            ng = len(states)
            # last three groups share one output buffer and one final DMA
            # (each dma_start costs ~625ns of serialized HWDGE generation,
            # which lands squarely on the drain tail)
            TAIL_T0 = states[-2]["T0"]
            NTAIL = TILES - TAIL_T0
            vtail = wpool.tile([P, NTAIL * D], F32, tag="vtail", name="vt")
            for s in states[-2:]:
                s["vtail_off"] = s["T0"] - TAIL_T0
            # head order: xh(g0), W chunk 0, xl(g0), W chunks 1-3 — each
            # dma_start costs ~625ns of serial HWDGE generation and the
            # transfers themselves serialize, so dependency order matters
            st0 = states[0]
            T0g, GS0 = st0["T0"], st0["GS"]
            xgh0 = st0["xgh"] = xpool.tile([P, GS0 * K], F16, tag="xgh",
                                           name="xgh")
            nc.sync.dma_start(
                xgh0[:].rearrange("p (t f) -> p t f", t=GS0),
                xTh[T0g:T0g + GS0].rearrange("t p f -> p t f"))
            wc0 = wpool.tile([P, WPK], F16, tag="wall0", name="wc")
            nc.sync.dma_start(wc0[:], Wpk.ap()[0])
            wchunks.append(wc0)
            xgl0 = st0["xgl"] = xpool.tile([P, GS0 * K], F16, tag="xgl",
                                           name="xgl")
            nc.sync.dma_start(
                xgl0[:].rearrange("p (t f) -> p t f", t=GS0),
                xTl[T0g:T0g + GS0].rearrange("t p f -> p t f"))
            for c in range(1, KCH):
                wc = wpool.tile([P, WPK], F16, tag=f"wall{c}", name="wc")
                nc.sync.dma_start(wc[:], Wpk.ap()[c])
                wchunks.append(wc)
            issue_dma(states[1])
            issue_dma(states[2])
            Wh = [wchunks[c][:, 0:ND] for c in range(KCH)]
            Wsh = [wchunks[c][:, ND:ND + D] for c in range(KCH)]
            Wl = [wchunks[c][:, ND + D:2 * ND + D] for c in range(KCH)]
            Wsl = [wchunks[c][:, 2 * ND + D:2 * ND + 2 * D] for c in range(KCH)]

            # --- PE pstate warm-up: junk matmuls on zeroed SBUF keep the
            # TensorEngine continuously busy through the initial DMA wait,
            # so the first real matmuls run at full clock ---
            wu_in = wpool.tile([P, P], F16, tag="wu_in", name="wu_in")
            wu_w = wpool.tile([P, 2 * P], F16, tag="wu_w", name="wu_w")
            nc.gpsimd.memset(wu_in[:], 0.0)
            nc.gpsimd.memset(wu_w[:], 0.0)
            wu_ps = upsum.tile([P, ND], F32, tag="u_ps", name="wu_ps")
            NWU = 12
            for r in range(NWU):
                nc.tensor.matmul(wu_ps[:, 0:2 * P], wu_in[:], wu_w[:],
                                 start=(r == 0), stop=(r == NWU - 1))

            # --- prefix-buffer ring: lead column stays 0 forever ---
            pslots = []
            for i in range(NSLOT):
                pt = prefpool.tile([P, ND + 1], F32, tag=f"pref{i}")
                nc.vector.memset(pt[:, 0:1], 0.0)
                pslots.append(pt)
            pctr = [0]

            def scan_pass(u_ap, in1_ap, order):
                """One fused MAC+prefix pass over u; returns (minuend,
                subtrahend) strided views whose difference is the segment
                sums (16 q-values for order='q', 32 s-values for 's')."""
                pt = pslots[pctr[0] % NSLOT]
                pctr[0] += 1
                if order == "q":
                    out_ap = pt[:, 1:ND + 1].rearrange("p (k d) -> p k d", d=D)
                    in0_ap = u_ap.rearrange("p (k d) -> p k d", d=D)
                    step = D
                    nseg = CAPS
                else:
                    out_ap = pt[:, 1:ND + 1].rearrange("p (d k) -> p d k", k=CAPS)
                    in0_ap = u_ap.rearrange("p (k d) -> p d k", d=D)
                    step = CAPS
                    nseg = D
                nc.vector._custom_dve(
                    mac_scan, out=out_ap, in0=in0_ap, in1=in1_ap)
                flat = pt[:].rearrange("p a -> p a")
                minu = flat[:, step::step]
                subt = flat[:, 0::step][:, 0:nseg]
                return minu, subt

            def alloc(st):
                GS = st["GS"]
                for tag, width in (
                    ("q1g", CAPS), ("yg", CAPS), ("e2g", CAPS),
                    ("q2g", CAPS), ("zg", CAPS), ("e3g", CAPS),
                    ("s3g", D), ("sqg", D),
                ):
                    st[tag] = gpool.tile([P, GS * width], F32, tag=tag, name=tag)
                off = st.get("vtail_off")
                if off is None:
                    st["vga"] = gpool.tile([P, GS * D], F32, tag="vg",
                                           name="vg")[:]
                else:
                    st["vga"] = vtail[:, off * D:(off + GS) * D]
                for tag in ("nu1", "m2q", "gam1", "r2g", "sig2", "nu2", "del2",
                            "m3z", "r3g", "sig3", "nu3", "alp3",
                            "tmpa", "tmpb"):
                    st[tag] = gpool.tile([P, GS], F32, tag=tag, name=tag)
                st["prodg"] = gpool.tile([P, GS * CAPS], F32, tag="prodg",
                                         name="prodg")
                st["u_tiles"] = []
                st["s1_tiles"] = []

            def phase1_tile(st, t):
                """matmuls + u/s1 copies + q1 scan (from PSUM) + diff."""
                xgh, xgl = st["xgh"], st["xgl"]
                u_ps = upsum.tile([P, ND], F32, tag="u_ps")
                s_ps = spsum.tile([P, D], F32, tag="s_ps")
                # u += xh@Wh + xh@Wl + xl@Wh   (fp16 hi/lo split); all
                # xh terms first so tile 0 can start before xl lands
                for c in range(KCH):
                    xh = xgh[:, t * K + c * P: t * K + (c + 1) * P]
                    first = c == 0
                    nc.tensor.matmul(u_ps[:], xh, Wh[c],
                                     start=first, stop=False)
                    nc.tensor.matmul(u_ps[:], xh, Wl[c],
                                     start=False, stop=False)
                    nc.tensor.matmul(s_ps[:], xh, Wsh[c],
                                     start=first, stop=False)
                    nc.tensor.matmul(s_ps[:], xh, Wsl[c],
                                     start=False, stop=False)
                for c in range(KCH):
                    xl = xgl[:, t * K + c * P: t * K + (c + 1) * P]
                    last = c == KCH - 1
                    nc.tensor.matmul(u_ps[:], xl, Wh[c],
                                     start=False, stop=last)
                    nc.tensor.matmul(s_ps[:], xl, Wsh[c],
                                     start=False, stop=last)

                s1_sb = s1pool.tile([P, D], F32, tag="s1_sb")
                nc.scalar.copy(s1_sb[:], s_ps[:])
                # nu1 = ||s1||^2 for free on Act: Square + accumulator; this
                # makes gamma1 computable before the q1 scans even finish
                s1q = s1pool.tile([P, D], F32, tag="s1q")
                nc.scalar.activation(s1q[:], s_ps[:], FN.Square,
                                     accum_out=st["nu1"][:, t:t + 1])
                u_sb = upool.tile([P, ND], F32, tag="u_sb")
                nc.scalar.copy(u_sb[:], u_ps[:])
                st["u_tiles"].append(u_sb)
                st["s1_tiles"].append(s1_sb)

                # q1 = sum_d u * bcast_k(s1): scan straight out of PSUM so it
                # does not wait for the SBUF copy
                minu, subt = scan_pass(
                    u_sb[:],
                    s1_sb[:].unsqueeze(1).broadcast_to([P, CAPS, D]),
                    "q")
                nc.gpsimd.tensor_tensor(
                    st["q1g"][:, t * CAPS:(t + 1) * CAPS], minu, subt, OP_SUB)

            def phase2_q2(st, t):
                """q2' scan + diff + e2*q2' product for tile t (skewed one
                tile behind the s2' scans so the Pool diff latency hides)."""
                u_sb = st["u_tiles"][t]
                e2s = st["e2g"][:, t * CAPS:(t + 1) * CAPS]
                s2p = st["s2p_tiles"][t]
                minu, subt = scan_pass(
                    u_sb[:],
                    s2p[:].unsqueeze(1).broadcast_to([P, CAPS, D]),
                    "q")
                q2s = st["q2g"][:, t * CAPS:(t + 1) * CAPS]
                nc.gpsimd.tensor_tensor(q2s, minu, subt, OP_SUB)
                nc.gpsimd.tensor_tensor(
                    st["prodg"][:, t * CAPS:(t + 1) * CAPS], q2s, e2s, OP_MUL)

            SKEW = 1   # q2' trails s2' by one tile (hides the Pool diff)

            def phase2_tile(st, t):
                """s2' scan for tile t, then the skewed q2' for t-SKEW."""
                if t == 0:
                    st["s2p_tiles"] = []
                u_sb = st["u_tiles"][t]
                e2s = st["e2g"][:, t * CAPS:(t + 1) * CAPS]
                minu, subt = scan_pass(
                    u_sb[:],
                    e2s.unsqueeze(1).broadcast_to([P, D, CAPS]),
                    "s")
                s2p = spool.tile([P, D], F32, tag="s2p")
                nc.gpsimd.tensor_tensor(s2p[:], minu, subt, OP_SUB)
                st["s2p_tiles"].append(s2p)
                if t >= SKEW:
                    phase2_q2(st, t - SKEW)

            def phase3_tile(st, t):
                u_sb = st["u_tiles"][t]
                e3s = st["e3g"][:, t * CAPS:(t + 1) * CAPS]
                minu, subt = scan_pass(
                    u_sb[:],
                    e3s.unsqueeze(1).broadcast_to([P, D, CAPS]),
                    "s")
                s3s = st["s3g"][:, t * D:(t + 1) * D]
                nc.gpsimd.tensor_tensor(s3s, minu, subt, OP_SUB)
                # sig3_t = ||s3'||^2 on Act (Square + accumulator)
                nc.scalar.activation(
                    st["sqg"][:, t * D:(t + 1) * D], s3s, FN.Square,
                    accum_out=st["sig3"][:, t:t + 1])

            # --- beta chains, split into window-spaced steps so no engine
            # ever waits in-order on a cross-engine round trip: each step's
            # inputs were produced at least one ~2.7us window earlier ---
            pending2 = []   # (st, t) phase2 tiles whose e2 exists
            pending3 = []   # (st, t) phase3 tiles whose e3 exists
            pendingB = []   # deferred beta-step closures

            def b1s1(st):
                # gamma1 = exp(0.5*ln(nu1)) / (1 + nu1); nu1 = ||s1||^2 was
                # accumulated per tile on Act, so this chain has no
                # dependence on the q1 scans at all
                nu1 = st["nu1"]
                nc.scalar.activation(st["tmpa"][:], nu1[:], FN.Ln)
                nc.scalar.activation(st["tmpa"][:], st["tmpa"][:], FN.Exp,
                                     scale=0.5)

            def b1s2(st):
                GS = st["GS"]
                q1g, yg, e2g = st["q1g"], st["yg"], st["e2g"]
                gam1, tmpa, tmpb = st["gam1"], st["tmpa"], st["tmpb"]
                q1v = q1g[:].rearrange("p (t k) -> p t k", t=GS)
                m2q = st["m2q"]
                nc.vector.tensor_reduce(m2q[:], q1v, AX, OP_MAX)
                # d2 = q1 - m2q (in place; gamma-independent since gam1 >= 0)
                nc.gpsimd.tensor_tensor(
                    q1v, q1v,
                    m2q[:].unsqueeze(2).broadcast_to([P, GS, CAPS]),
                    OP_SUB,
                )
                nc.vector.tensor_scalar(tmpb[:], st["nu1"][:], 1.0, 1.0,
                                        OP_MUL, OP_ADD)
                nc.vector.reciprocal(tmpb[:], tmpb[:])
                nc.vector.tensor_tensor(gam1[:], tmpa[:], tmpb[:], OP_MUL)
                for t in range(GS):
                    nc.scalar.activation(
                        e2g[:, t * CAPS:(t + 1) * CAPS],
                        q1g[:, t * CAPS:(t + 1) * CAPS],
                        FN.Exp, scale=gam1[:, t:t + 1])
                # y = l2 - gam1*m2q, carried into iteration 3's logits
                nc.gpsimd.tensor_tensor(
                    yg[:].rearrange("p (t k) -> p t k", t=GS),
                    q1g[:].rearrange("p (t k) -> p t k", t=GS),
                    gam1[:].unsqueeze(2).broadcast_to([P, GS, CAPS]),
                    OP_MUL,
                )
                pending2.extend((st, t) for t in range(GS))

            def b2s1(st):
                GS = st["GS"]
                sig2, nu2, r2g = st["sig2"], st["nu2"], st["r2g"]
                tmpa = st["tmpa"]
                nc.vector.tensor_reduce(
                    r2g[:], st["e2g"][:].rearrange("p (t k) -> p t k", t=GS),
                    AX, OP_ADD)
                nc.vector.reciprocal(r2g[:], r2g[:])
                nc.vector.tensor_reduce(
                    sig2[:], st["prodg"][:].rearrange("p (t k) -> p t k", t=GS),
                    AX, OP_ADD)
                nc.vector.tensor_tensor(tmpa[:], r2g[:], r2g[:], OP_MUL)
                nc.vector.tensor_tensor(nu2[:], sig2[:], tmpa[:], OP_MUL)
                nc.scalar.activation(tmpa[:], nu2[:], FN.Ln)
                nc.scalar.activation(tmpa[:], tmpa[:], FN.Exp, scale=0.5)

            def b2s2(st):
                GS = st["GS"]
                tmpa, tmpb, del2 = st["tmpa"], st["tmpb"], st["del2"]
                prodg, q2g, zg, yg = (st["prodg"], st["q2g"], st["zg"],
                                      st["yg"])
                nc.vector.tensor_scalar(tmpb[:], st["nu2"][:], 1.0, 1.0,
                                        OP_MUL, OP_ADD)
                nc.vector.reciprocal(tmpb[:], tmpb[:])
                nc.vector.tensor_tensor(tmpa[:], tmpa[:], tmpb[:], OP_MUL)
                nc.vector.tensor_tensor(del2[:], tmpa[:], st["r2g"][:], OP_MUL)
                # z = y + del2*q2' = l3 - gam1*m2q
                nc.gpsimd.tensor_tensor(
                    prodg[:].rearrange("p (t k) -> p t k", t=GS),
                    q2g[:].rearrange("p (t k) -> p t k", t=GS),
                    del2[:].unsqueeze(2).broadcast_to([P, GS, CAPS]),
                    OP_MUL,
                )
                nc.gpsimd.tensor_tensor(zg[:], yg[:], prodg[:], OP_ADD)

            def b2s3(st):
                GS = st["GS"]
                zg, e3g, m3z = st["zg"], st["e3g"], st["m3z"]
                zv = zg[:].rearrange("p (t k) -> p t k", t=GS)
                nc.vector.tensor_reduce(m3z[:], zv, AX, OP_MAX)
                nc.gpsimd.tensor_tensor(
                    zv, zv,
                    m3z[:].unsqueeze(2).broadcast_to([P, GS, CAPS]),
                    OP_SUB,
                )
                for t in range(GS):
                    nc.scalar.activation(
                        e3g[:, t * CAPS:(t + 1) * CAPS],
                        zg[:, t * CAPS:(t + 1) * CAPS],
                        FN.Exp)
                pending3.extend((st, t) for t in range(GS))

            def b3s1(st):
                GS = st["GS"]
                sig3, nu3, r3g = st["sig3"], st["nu3"], st["r3g"]
                tmpa = st["tmpa"]
                nc.vector.tensor_reduce(
                    r3g[:], st["e3g"][:].rearrange("p (t k) -> p t k", t=GS),
                    AX, OP_ADD)
                nc.vector.reciprocal(r3g[:], r3g[:])
                nc.vector.tensor_tensor(tmpa[:], r3g[:], r3g[:], OP_MUL)
                nc.vector.tensor_tensor(nu3[:], sig3[:], tmpa[:], OP_MUL)
                nc.scalar.activation(tmpa[:], nu3[:], FN.Ln)
                nc.scalar.activation(tmpa[:], tmpa[:], FN.Exp, scale=0.5)

            def b3s2(st):
                GS, T0 = st["GS"], st["T0"]
                tmpa, tmpb, alp3 = st["tmpa"], st["tmpb"], st["alp3"]
                s3g, vga = st["s3g"], st["vga"]
                nc.vector.tensor_scalar(tmpb[:], st["nu3"][:], 1.0, 1.0,
                                        OP_MUL, OP_ADD)
                nc.vector.reciprocal(tmpb[:], tmpb[:])
                nc.vector.tensor_tensor(tmpa[:], tmpa[:], tmpb[:], OP_MUL)
                nc.vector.tensor_tensor(alp3[:], tmpa[:], st["r3g"][:], OP_MUL)
                nc.gpsimd.tensor_tensor(
                    vga.rearrange("p (t d) -> p t d", t=GS),
                    s3g[:].rearrange("p (t d) -> p t d", t=GS),
                    alp3[:].unsqueeze(2).broadcast_to([P, GS, D]),
                    OP_MUL,
                )
                if st.get("vtail_off") is None:
                    nc.sync.dma_start(
                        vview[T0:T0 + GS].rearrange("t p d -> p t d"),
                        vga.rearrange("p (t d) -> p t d", t=GS))

            def do_phase2(item):
                st2, t2 = item
                phase2_tile(st2, t2)
                st2["p2done"] = st2.get("p2done", 0) + 1
                if st2["p2done"] == st2["GS"]:
                    for tt in range(max(0, st2["GS"] - SKEW), st2["GS"]):
                        phase2_q2(st2, tt)
                    pendingB.append((id(st2), lambda s=st2: b2s1(s)))
                    pendingB.append((id(st2), lambda s=st2: b2s2(s)))
                    pendingB.append((id(st2), lambda s=st2: b2s3(s)))

            def do_phase3(item):
                st3, t3 = item
                phase3_tile(st3, t3)
                st3["p3done"] = st3.get("p3done", 0) + 1
                if st3["p3done"] == st3["GS"]:
                    pendingB.append((id(st3), lambda s=st3: b3s1(s)))
                    pendingB.append((id(st3), lambda s=st3: b3s2(s)))

            def pump(n=2):
                for _ in range(n):
                    if pendingB:
                        pendingB.pop(0)[1]()

            # ---- global-queue software pipeline: every matmul window also
            # hosts one phase2 tile (2 scans), one phase3 tile (1 scan) and
            # up to two deferred beta steps ----
            for g in range(ng):
                st = states[g]
                if g >= 1 and g + 2 < ng:
                    issue_dma(states[g + 2])
                alloc(st)
                for t in range(st["GS"]):
                    pump()
                    if pending2:
                        do_phase2(pending2.pop(0))
                    if pending3:
                        do_phase3(pending3.pop(0))
                    phase1_tile(st, t)
                b1s1(st)
                pendingB.append((id(st), lambda s=st: b1s2(s)))
            # ---- drain ----
            while pendingB or pending2 or pending3:
                pump()
                if pending2:
                    do_phase2(pending2.pop(0))
                if pending3:
                    do_phase3(pending3.pop(0))
            nc.sync.dma_start(
                vview[TAIL_T0:TILES].rearrange("t p d -> p t d"),
                vtail[:].rearrange("p (t d) -> p t d", t=NTAIL))

    nc.compile()
    return nc


_PROG_CACHE = {}


def _get_program():
    if "nc" not in _PROG_CACHE:
        _PROG_CACHE["nc"] = _build_program()
    return _PROG_CACHE["nc"]


def _split16(a):
    hi = a.astype(np.float16)
    lo = (a - hi.astype(np.float32)).astype(np.float16)
    return hi, lo


def _stage_inputs(x, W):
    x = np.ascontiguousarray(x, dtype=np.float32)
    W = np.ascontiguousarray(W, dtype=np.float32)
    Ws = W.reshape(K, CAPS, D).mean(axis=1, dtype=np.float32)
    Whh, Wll = _split16(W.reshape(KCH, P, ND))
    Wsh, Wsl = _split16(Ws.reshape(KCH, P, D))
    Wpk = np.ascontiguousarray(
        np.concatenate([Whh, Wsh, Wll, Wsl], axis=2))

    in_maps = []
    for core in range(NCORES):
        xs = x[core * BS:(core + 1) * BS]
        # lhsT tile layout: [tile, kappa_in_chunk(P), (chunk, j)]
        xt = np.ascontiguousarray(
            xs.reshape(TILES, P, KCH, P).transpose(0, 3, 2, 1)
        ).reshape(TILES, P, K)
        xh, xl = _split16(xt)
        in_maps.append({"xh": xh, "xl": xl, "Wpk": Wpk})
    return in_maps


def kernel(x, W, _trace=False, _trace_kwargs=None):
    nc = _get_program()
    in_maps = _stage_inputs(np.asarray(x), np.asarray(W))
    res = run_bass_kernel_spmd(
        nc, in_maps, list(range(NCORES)), trace=_trace,
        **(_trace_kwargs or {}),
    )
    out = np.concatenate(
        [np.asarray(res.results[i]["v"], dtype=np.float32) for i in range(NCORES)],
        axis=0,
    )
    if _trace:
        kernel._last_results = res
    return out
